# revision 17
# baseline (speedup 1.0000x reference)
"""Butterfly-Conv2d (nn_BConv2d) Trainium2 kernel — v5.

Math (reference): x(B=64,IC=16,32,32) -> y=x.reshape(IC,B,N=1024)[:,:,bitrev];
broadcast over OC=32; 10 radix-2 butterfly layers with per-(ic,oc) twiddles;
mean over ic; + bias -> (B,OC,32,32).

Strategy:
  * Shard over OC: 8 cores x 4 oc each; every core holds all 16 ic so the
    ic-mean is core-local (no collective). Host concatenates oc slices.
  * Stage A (TensorE, bf16): butterfly layers 0..7 compose into dense
    256x256 block-diagonal matrices; K=128 matmuls with PSUM K-accumulation
    (16 MMs per (ic,oc), N=64). MMs pipeline at ~53 ns.
  * Ratio folding: for layers 8 and 9 the "low" output of each butterfly
    pair is made coefficient-free by folding its twiddle (and the 1/IC
    mean) into the stage-A matrix rows on the host; the "high" output uses
    host-computed coefficient RATIOS (bf16's 8-bit exponent keeps the
    Cauchy-tailed ratios safe — errors stay relative).
  * Stage B per (ic,oc) is four 2x-mode flat [128,512]/[128,256] DVE ops
    (one add + two coefficient mults + one add); the per-chunk coefficient
    vectors are materialized once per pair into flat tiles by ScalarE /
    GpSimd via stride-0 broadcast access patterns.
  * The remaining butterfly adds AND the ic-mean happen on TensorE: four
    identity matmuls per pair accumulate y8/u chunk-views straight into a
    per-oc PSUM accumulator (fp32, seeded by a bias matmul). These acc-MMs
    are issued DELAYED by 2 pairs so the in-order PE never stalls waiting
    for the current pair's vector chain.

Device layout: z/zc/y8/m/u tiles [128, (q8|p8, n9, n7, b)] with partition
p = n&127 (chunk-block order n8-major); acc [128, (p9, n8, n7, b)], which
makes the output chunk order natural.
"""

import numpy as np
import ml_dtypes

B, IC, OC, H, W = 64, 16, 32, 32, 32
N = H * W          # 1024
M = 10             # butterfly layers
NCORES = 8
OCL = OC // NCORES  # 4 oc per core
NCH = 8            # free-dim chunks (n9n8n7)
P = 128            # partitions (n6..n0)
SB = 256           # composed stage-A block size (layers 0..7)
NBLK = N // SB     # 4 blocks per (ic,oc)
WCOLS = NBLK * 4 * P      # 2048 weight cols per (ic,oc)
TBCOLS = 16               # ratio-table cols appended to w

W_DTYPE = ml_dtypes.bfloat16

# engine per stage-B op: 'V' = DVE, 'G' = GpSimd, 'S' = ScalarE.
# NOTE: GpSimd shares VectorE's SBUF port — concurrent GpSimd traffic
# slows DVE ops ~4x, so GpSimd is left idle on purpose.
ENG = {
    "zcopy": "S",      # zc = bf16(z), PSUM->SBUF [128,512]
    "rs_mat": "S",     # rs = bcast(tb cols 0..7)   [128,512]
    "l8low": "V",      # y8_low = zc0 + zc1          [128,256] add 2x
    "m_mult": "V",     # m = zc * rs                 [128,512] mult 2x
    "l8hi_add": "V",   # y8_hi = m0 + m1             [128,256] add 2x
    "u_mult": "V",     # u = y8 * (rA|rB)-bcast      [128,512] mult 1x
}
DELAY_PAIRS = 3        # acc-MM issue delay (pairs) to keep PE un-stalled
ACC_MM_MODE = "mm2"    # "mm2": stride-0-out accumulate MMs; "mm4": 4 plain MMs


def _bitrev(n):
    bits = int(np.log2(n))
    idx = np.arange(n, dtype=np.int64)
    rev = np.zeros(n, dtype=np.int64)
    for b in range(bits):
        rev = (rev << 1) | ((idx >> b) & 1)
    return rev


def _compose_stageA(tw):
    """Compose butterfly layers 0..7 into A[ic,oc,g,256,256] (g=(n9,n8))."""
    ic, oc = tw.shape[0], tw.shape[1]
    A = np.zeros((ic, oc, NBLK, SB, SB), dtype=np.float64)
    eye = np.eye(SB, dtype=np.float64)
    A[:] = eye
    for l in range(8):
        s = 1 << l
        nb_loc = SB // (2 * s)
        t = tw[:, :, l].reshape(ic, oc, N // (2 * s), s, 2, 2)
        t = t.reshape(ic, oc, NBLK, nb_loc, s, 2, 2).astype(np.float64)
        Av = A.reshape(ic, oc, NBLK, nb_loc, 2, s, SB)
        a0 = Av[:, :, :, :, 0].copy()
        a1 = Av[:, :, :, :, 1].copy()
        t00 = t[..., 0, 0, None]
        t01 = t[..., 0, 1, None]
        t10 = t[..., 1, 0, None]
        t11 = t[..., 1, 1, None]
        Av[:, :, :, :, 0] = t00 * a0 + t01 * a1
        Av[:, :, :, :, 1] = t10 * a0 + t11 * a1
    return A


def _fold_and_ratios(tw):
    """Row scalings for stage A plus stage-B ratio tables (float64).

    Returns (S, tb):
      S[ic, oc, n9, n8, n7, p]  row scaling for z-chunk (n9,n8,n7)
      tb[ic, oc, 16, p]  cols 0..7: (r|s) for L8-high, col = q8*4+n9*2+n7
                         cols 8..15: (rA|rB) for L9-high, col = 8+p8*4+q9*2+n7
    """
    ic, oc = tw.shape[0], tw.shape[1]
    # t8[k=n9, j(n7,p), p_out8, q8] with j in [0,256)
    t8 = tw[:, :, 8].reshape(ic, oc, 2, 256, 2, 2).astype(np.float64)
    t9 = tw[:, :, 9].reshape(ic, oc, 512, 2, 2).astype(np.float64)
    pr = np.arange(P)
    S = np.zeros((ic, oc, 2, 2, 2, P), dtype=np.float64)
    tb = np.zeros((ic, oc, TBCOLS, P), dtype=np.float64)
    for n9 in range(2):
        for n8 in range(2):
            for n7 in range(2):
                j = n7 * 128 + pr
                S[:, :, n9, n8, n7] = (
                    t8[:, :, n9, j, 0, n8] * t9[:, :, j, 0, n9] / IC
                )
    for n9 in range(2):
        for n7 in range(2):
            q = n9 * 2 + n7
            j = n7 * 128 + pr
            g_even = t9[:, :, 0 * 256 + j, 0, n9]
            g_odd = t9[:, :, 1 * 256 + j, 0, n9]
            tb[:, :, q] = (
                t8[:, :, n9, j, 1, 0] * g_odd
            ) / (t8[:, :, n9, j, 0, 0] * g_even)
            tb[:, :, 4 + q] = (
                t8[:, :, n9, j, 1, 1] * g_odd
            ) / (t8[:, :, n9, j, 0, 1] * g_even)
    for n8 in range(2):
        for q9 in range(2):
            for n7 in range(2):
                j9 = n8 * 256 + n7 * 128 + pr
                tb[:, :, 8 + n8 * 4 + q9 * 2 + n7] = (
                    t9[:, :, j9, 1, q9] / t9[:, :, j9, 0, q9]
                )
    return S, tb


def _prep_host(x, twiddle, bias):
    """All host-side layout work. Returns per-core input maps (numpy)."""
    perm = _bitrev(N)
    y = np.ascontiguousarray(x).reshape(IC, B, N)[:, :, perm]
    # device layout y[ic, p, c*64+b], chunk-major c=(n9,n8,n7)
    y_dev = np.ascontiguousarray(
        y.reshape(IC, B, NCH, P).transpose(0, 3, 2, 1)
    ).reshape(IC, P, NCH * B)

    tw = np.asarray(twiddle, dtype=np.float64)
    A = _compose_stageA(tw)
    S, tb = _fold_and_ratios(tw)
    # scale A rows: block g=(n9,n8), row (n7out*128+p) *= S[n9,n8,n7out,p]
    Av = A.reshape(IC, OC, 2, 2, 2, P, SB)  # [n9, n8, n7out, p, col]
    Av *= S[..., None]

    bias_np = np.asarray(bias, dtype=np.float64).reshape(OC, NCH, P)

    in_maps = []
    for core in range(NCORES):
        osl = slice(core * OCL, (core + 1) * OCL)
        Ac = A[:, osl]  # (IC, OCL, 4, 256, 256) float64
        # lhsT tiles: w[ic,o,p_k, g, h, kin, m] = Ac[ic,o,g][h*128+m, kin*128+p_k]
        w = np.ascontiguousarray(
            Ac.reshape(IC, OCL, NBLK, 2, P, 2, P)  # [g, h, m, kin, k]
            .transpose(0, 1, 6, 2, 3, 5, 4)        # [ic,o,k,g,h,kin,m]
        ).astype(W_DTYPE).reshape(IC, OCL, P, WCOLS)
        tbc = np.ascontiguousarray(
            tb[:, osl].transpose(0, 1, 3, 2)  # [ic,o,p,16]
        ).astype(W_DTYPE)
        wcat = np.concatenate([w, tbc], axis=3)  # [ic,o,p,2064]
        # bias in acc layout (p9, n8, n7) = natural chunk order
        bc = np.ascontiguousarray(
            np.broadcast_to(
                bias_np[osl].transpose(0, 2, 1)[:, :, :, None],
                (OCL, P, NCH, B),
            )
        ).reshape(OCL, P, NCH * B).astype(W_DTYPE)
        in_maps.append(
            {
                "y": y_dev.astype(W_DTYPE),
                "w": wcat,
                "bias": bc,
                "eye": np.eye(P, dtype=np.float32).astype(W_DTYPE),
            }
        )
    return in_maps


def _emulate_core(im):
    """Numpy emulation mirroring the device program (incl. bf16 rounding)."""
    f32 = np.float32

    def rt(a):  # round-trip through W_DTYPE
        return a.astype(W_DTYPE).astype(f32)

    y = im["y"].astype(f32)             # (IC, 128, 512)
    wall = im["w"]
    w = wall[..., :WCOLS].astype(f32).reshape(IC, OCL, P, NBLK, 2, 2, P)
    tb = wall[..., WCOLS:].astype(f32)  # (IC, OCL, P, 16) bf16 values
    acc = im["bias"].astype(f32).reshape(OCL, P, 2, 2, 2, B).copy()
    for o in range(OCL):
        for ic in range(IC):
            z = np.zeros((P, 2, 2, 2, B), dtype=f32)  # [p, n8, n9, n7, b]
            yv = y[ic].reshape(P, NCH, B)  # chunks (n9,n8,n7)
            for g in range(NBLK):          # g = (n9, n8)
                n9, n8 = g >> 1, g & 1
                for h in range(2):         # n7out
                    a = np.zeros((P, B), dtype=f32)
                    for kin in range(2):
                        lhsT = w[ic, o, :, g, h, kin]  # [k, m]
                        a += lhsT.T @ yv[:, 2 * g + kin]
                    z[:, n8, n9, h] = a
            zc = rt(z)
            # L8: y8[p8, n9, n7]
            y8 = np.zeros_like(zc)
            y8[:, 0] = rt(zc[:, 0] + zc[:, 1])
            mm = rt(zc * tb[ic, o, :, :8].reshape(P, 2, 2, 2, 1))
            y8[:, 1] = rt(mm[:, 0] + mm[:, 1])
            # L9: u = y8 * (rA|rB)
            u = rt(y8 * tb[ic, o, :, 8:].reshape(P, 2, 2, 2, 1))
            # identity matmuls accumulate into fp32 acc [p9, n8, n7]
            acc[o, :, 0] += y8[:, :, 0] + y8[:, :, 1]
            acc[o, :, 1] += u[:, :, 0] + u[:, :, 1]
    return acc.reshape(OCL, P, NCH * B)


def _build_program():
    import concourse.bacc as bacc
    import concourse.mybir as mybir
    from concourse.tile import TileContext

    wdt = mybir.dt.bfloat16 if W_DTYPE != np.float32 else mybir.dt.float32
    f32 = mybir.dt.float32
    ADD, MULT = mybir.AluOpType.add, mybir.AluOpType.mult

    nc = bacc.Bacc(None, target_bir_lowering=False)
    y_d = nc.dram_tensor("y", (IC, P, NCH * B), wdt, kind="ExternalInput")
    w_d = nc.dram_tensor(
        "w", (IC, OCL, P, WCOLS + TBCOLS), wdt, kind="ExternalInput"
    )
    bias_d = nc.dram_tensor("bias", (OCL, P, NCH * B), wdt, kind="ExternalInput")
    eye_d = nc.dram_tensor("eye", (P, P), wdt, kind="ExternalInput")
    o_d = nc.dram_tensor("o", (OCL, P, NCH * B), f32, kind="ExternalOutput")

    with TileContext(nc) as tc:
        with (
            tc.tile_pool(name="ypool", bufs=2) as ypool,
            tc.tile_pool(name="wpool", bufs=3) as wpool,
            tc.tile_pool(name="const", bufs=1) as cpool,
            tc.tile_pool(name="zcpool", bufs=3) as zcpool,
            tc.tile_pool(name="y8pool", bufs=2 + DELAY_PAIRS) as y8pool,
            tc.tile_pool(name="mpool", bufs=2) as mpool,
            tc.tile_pool(name="upool", bufs=2 + DELAY_PAIRS) as upool,
            tc.tile_pool(name="rspool", bufs=3) as rspool,
            tc.tile_pool(name="opool", bufs=OCL) as opool,
            tc.tile_pool(name="zps", bufs=3, space="PSUM") as zps,
            tc.tile_pool(name="accps", bufs=OCL, space="PSUM") as accps,
        ):
            engs = {"V": nc.vector, "G": nc.gpsimd, "S": nc.scalar}

            def vec_copy(engine, out, in_):
                if engine == "S":
                    nc.scalar.copy(out, in_)
                else:
                    engs[engine].tensor_copy(out, in_)

            eye = cpool.tile([P, P], wdt, tag="eye")
            nc.sync.dma_start(out=eye[:], in_=eye_d[:, :])
            accs = []
            for o in range(OCL):
                bt = cpool.tile([P, NCH * B], wdt, tag=f"bias{o}")
                nc.sync.dma_start(out=bt[:], in_=bias_d[o])
                acc = accps.tile([P, NCH * B], f32, tag="acc")
                nc.tensor.matmul(
                    acc[:], eye[:], bt[:],
                    start=True, stop=False, skip_group_check=True,
                )
                accs.append(acc)

            pending = []

            def flush_one():
                o, y8t, ut, last = pending.pop(0)
                hB = NCH * B // 2
                split = "p (a q c b) -> p a q c b"
                if ACC_MM_MODE == "mm2":
                    # q9-sum via PSUM accumulate of a stride-0 out AP
                    for tile_, lo in ((y8t, True), (ut, False)):
                        rhs = tile_[:].rearrange(split, a=2, q=2, c=2)
                        base = accs[o][:, 0:hB] if lo else accs[o][:, hB:]
                        outap = (
                            base.rearrange("p (a c b) -> p a c b", a=2, c=2)
                            .unsqueeze(2)
                            .broadcast_to([P, 2, 2, 2, B])
                        )
                        nc.tensor.matmul(
                            outap,
                            eye[:],
                            rhs,
                            start=False,
                            stop=(last and not lo),
                            skip_group_check=True,
                        )
                else:
                    y8v = y8t[:].rearrange(split, a=2, q=2, c=2)
                    uv = ut[:].rearrange(split, a=2, q=2, c=2)
                    for k, rhs in enumerate(
                        (y8v[:, :, 0], y8v[:, :, 1], uv[:, :, 0], uv[:, :, 1])
                    ):
                        lo = k < 2
                        nc.tensor.matmul(
                            accs[o][:, 0:hB] if lo else accs[o][:, hB:],
                            eye[:],
                            rhs,
                            start=False,
                            stop=(last and k == 3),
                            skip_group_check=True,
                        )

            for ic in range(IC):
                ytile = ypool.tile([P, NCH * B], wdt)
                nc.sync.dma_start(out=ytile[:], in_=y_d[ic])
                for o in range(OCL):
                    wtile = wpool.tile([P, WCOLS + TBCOLS], wdt)
                    nc.sync.dma_start(out=wtile[:], in_=w_d[ic, o])
                    z = zps.tile([P, 2, 2, 2, B], f32)  # [n8, n9, n7, b]
                    for g in range(NBLK):
                        n9, n8 = g >> 1, g & 1
                        for h in range(2):
                            for kin in range(2):
                                wi = ((g * 2 + h) * 2 + kin) * P
                                nc.tensor.matmul(
                                    z[:, n8, n9, h],
                                    wtile[:, wi : wi + P],
                                    ytile[:, (2 * g + kin) * B : (2 * g + kin + 1) * B],
                                    start=(kin == 0),
                                    stop=(kin == 1),
                                    skip_group_check=True,
                                )
                    while len(pending) >= DELAY_PAIRS:
                        flush_one()
                    bcast8 = lambda c0: (
                        wtile[:, WCOLS + c0 : WCOLS + c0 + 8]
                        .unsqueeze(2)
                        .broadcast_to([P, 8, B])
                    )
                    rs = rspool.tile([P, NCH * B], wdt, tag="rs")
                    vec_copy(
                        ENG["rs_mat"],
                        rs[:].rearrange("p (c b) -> p c b", c=8),
                        bcast8(0),
                    )
                    zc = zcpool.tile([P, 2, 2, 2, B], wdt, tag="zc")
                    vec_copy(ENG["zcopy"], zc[:], z[:])
                    zcf = zc[:].rearrange("p a b c d -> p (a b c d)")
                    hB = NCH * B // 2
                    y8 = y8pool.tile([P, NCH * B], wdt)  # [(p8, n9, n7), b]
                    m = mpool.tile([P, NCH * B], wdt)    # [(q8, n9, n7), b]
                    u = upool.tile([P, NCH * B], wdt)    # [(p8, q9, n7), b]
                    engs[ENG["l8low"]].tensor_tensor(
                        y8[:, 0:hB], zcf[:, 0:hB], zcf[:, hB:], op=ADD
                    )
                    engs[ENG["m_mult"]].tensor_tensor(
                        m[:], zcf, rs[:], op=MULT
                    )
                    engs[ENG["l8hi_add"]].tensor_tensor(
                        y8[:, hB:], m[:, 0:hB], m[:, hB:], op=ADD
                    )
                    engs[ENG["u_mult"]].tensor_tensor(
                        u[:].rearrange("p (c b) -> p c b", c=8),
                        y8[:].rearrange("p (c b) -> p c b", c=8),
                        bcast8(8),
                        op=MULT,
                    )
                    pending.append((o, y8, u, ic == IC - 1))
            while pending:
                flush_one()
            for o in range(OCL):
                ot = opool.tile([P, NCH * B], f32, tag=f"out{o}")
                nc.scalar.copy(ot[:], accs[o][:])
                nc.sync.dma_start(out=o_d[o], in_=ot[:])
    nc.finalize()
    return nc


_LAST_RESULTS = {"exec_time_ns": None}


def kernel(x, twiddle, bias, _trace=False, _emulate=False):
    in_maps = _prep_host(np.asarray(x), np.asarray(twiddle), np.asarray(bias))
    if _emulate:
        outs = [_emulate_core(im) for im in in_maps]
    else:
        from concourse.bass_utils import run_bass_kernel_spmd

        nc = _build_program()
        res = run_bass_kernel_spmd(
            nc, in_maps, list(range(NCORES)), trace=_trace
        )
        _LAST_RESULTS["exec_time_ns"] = res.exec_time_ns
        _LAST_RESULTS["mean_exec_time_ns"] = res.mean_exec_time_ns
        outs = [r["o"] for r in res.results]
    # o[oc_l, p, c*64+b] with chunk c=(n9,n8,n7) natural; (OC,B,N) ->
    # (B,OC,H,W) is a pure reinterpret (reference uses .reshape).
    full = np.concatenate(
        [
            np.asarray(o, dtype=np.float32)
            .reshape(OCL, P, NCH, B)
            .transpose(0, 3, 2, 1)
            .reshape(OCL, B, N)
            for o in outs
        ],
        axis=0,
    )
    return np.ascontiguousarray(full).reshape(B, OC, H, W).astype(np.float32)


# revision 18
# speedup vs baseline: 1.5044x; 1.5044x over previous
"""Butterfly-Conv2d (nn_BConv2d) Trainium2 kernel — v5.

Math (reference): x(B=64,IC=16,32,32) -> y=x.reshape(IC,B,N=1024)[:,:,bitrev];
broadcast over OC=32; 10 radix-2 butterfly layers with per-(ic,oc) twiddles;
mean over ic; + bias -> (B,OC,32,32).

Strategy:
  * Shard over OC: 8 cores x 4 oc each; every core holds all 16 ic so the
    ic-mean is core-local (no collective). Host concatenates oc slices.
  * Stage A (TensorE, bf16): butterfly layers 0..7 compose into dense
    256x256 block-diagonal matrices; K=128 matmuls with PSUM K-accumulation
    (16 MMs per (ic,oc), N=64). MMs pipeline at ~53 ns.
  * Ratio folding: for layers 8 and 9 the "low" output of each butterfly
    pair is made coefficient-free by folding its twiddle (and the 1/IC
    mean) into the stage-A matrix rows on the host; the "high" output uses
    host-computed coefficient RATIOS (bf16's 8-bit exponent keeps the
    Cauchy-tailed ratios safe — errors stay relative).
  * Stage B per (ic,oc) is four 2x-mode flat [128,512]/[128,256] DVE ops
    (one add + two coefficient mults + one add); the per-chunk coefficient
    vectors are materialized once per pair into flat tiles by ScalarE /
    GpSimd via stride-0 broadcast access patterns.
  * The remaining butterfly adds AND the ic-mean happen on TensorE: four
    identity matmuls per pair accumulate y8/u chunk-views straight into a
    per-oc PSUM accumulator (fp32, seeded by a bias matmul). These acc-MMs
    are issued DELAYED by 2 pairs so the in-order PE never stalls waiting
    for the current pair's vector chain.

Device layout: z/zc/y8/m/u tiles [128, (q8|p8, n9, n7, b)] with partition
p = n&127 (chunk-block order n8-major); acc [128, (p9, n8, n7, b)], which
makes the output chunk order natural.
"""

import numpy as np
import ml_dtypes

B, IC, OC, H, W = 64, 16, 32, 32, 32
N = H * W          # 1024
M = 10             # butterfly layers
NCORES = 8
OCL = OC // NCORES  # 4 oc per core
NCH = 8            # free-dim chunks (n9n8n7)
P = 128            # partitions (n6..n0)
SB = 256           # composed stage-A block size (layers 0..7)
NBLK = N // SB     # 4 blocks per (ic,oc)
WCOLS = NBLK * 4 * P      # 2048 weight cols per (ic,oc)
TBCOLS = 16               # ratio-table cols appended to w

W_DTYPE = ml_dtypes.bfloat16

# engine per stage-B op: 'V' = DVE, 'G' = GpSimd, 'S' = ScalarE.
# NOTE: GpSimd shares VectorE's SBUF port — concurrent GpSimd traffic
# slows DVE ops ~4x, so GpSimd is left idle on purpose.
ENG = {
    "zcopy": "S",      # zc = bf16(z), PSUM->SBUF [128,512]
    "rs_mat": "S",     # rs = bcast(tb cols 0..7)   [128,512]
    "l8low": "V",      # y8_low = zc0 + zc1          [128,256] add 2x
    "m_mult": "V",     # m = zc * rs                 [128,512] mult 2x
    "l8hi_add": "V",   # y8_hi = m0 + m1             [128,256] add 2x
    "u_mult": "V",     # u = y8 * (rA|rB)-bcast      [128,512] mult 1x
}
DELAY_PAIRS = 3        # acc-MM issue delay (pairs) to keep PE un-stalled
ACC_MM_MODE = "mm2"    # "mm2": stride-0-out accumulate MMs; "mm4": 4 plain MMs


def _bitrev(n):
    bits = int(np.log2(n))
    idx = np.arange(n, dtype=np.int64)
    rev = np.zeros(n, dtype=np.int64)
    for b in range(bits):
        rev = (rev << 1) | ((idx >> b) & 1)
    return rev


def _compose_stageA(tw):
    """Compose butterfly layers 0..7 into A[ic,oc,g,256,256] (g=(n9,n8))."""
    ic, oc = tw.shape[0], tw.shape[1]
    A = np.zeros((ic, oc, NBLK, SB, SB), dtype=np.float64)
    eye = np.eye(SB, dtype=np.float64)
    A[:] = eye
    for l in range(8):
        s = 1 << l
        nb_loc = SB // (2 * s)
        t = tw[:, :, l].reshape(ic, oc, N // (2 * s), s, 2, 2)
        t = t.reshape(ic, oc, NBLK, nb_loc, s, 2, 2).astype(np.float64)
        Av = A.reshape(ic, oc, NBLK, nb_loc, 2, s, SB)
        a0 = Av[:, :, :, :, 0].copy()
        a1 = Av[:, :, :, :, 1].copy()
        t00 = t[..., 0, 0, None]
        t01 = t[..., 0, 1, None]
        t10 = t[..., 1, 0, None]
        t11 = t[..., 1, 1, None]
        Av[:, :, :, :, 0] = t00 * a0 + t01 * a1
        Av[:, :, :, :, 1] = t10 * a0 + t11 * a1
    return A


def _fold_and_ratios(tw):
    """Row scalings for stage A plus stage-B ratio tables (float64).

    Returns (S, tb):
      S[ic, oc, n9, n8, n7, p]  row scaling for z-chunk (n9,n8,n7)
      tb[ic, oc, 16, p]  cols 0..7: (r|s) for L8-high, col = q8*4+n9*2+n7
                         cols 8..15: (rA|rB) for L9-high, col = 8+p8*4+q9*2+n7
    """
    ic, oc = tw.shape[0], tw.shape[1]
    # t8[k=n9, j(n7,p), p_out8, q8] with j in [0,256)
    t8 = tw[:, :, 8].reshape(ic, oc, 2, 256, 2, 2).astype(np.float64)
    t9 = tw[:, :, 9].reshape(ic, oc, 512, 2, 2).astype(np.float64)
    pr = np.arange(P)
    S = np.zeros((ic, oc, 2, 2, 2, P), dtype=np.float64)
    tb = np.zeros((ic, oc, TBCOLS, P), dtype=np.float64)
    for n9 in range(2):
        for n8 in range(2):
            for n7 in range(2):
                j = n7 * 128 + pr
                S[:, :, n9, n8, n7] = (
                    t8[:, :, n9, j, 0, n8] * t9[:, :, j, 0, n9] / IC
                )
    for n9 in range(2):
        for n7 in range(2):
            q = n9 * 2 + n7
            j = n7 * 128 + pr
            g_even = t9[:, :, 0 * 256 + j, 0, n9]
            g_odd = t9[:, :, 1 * 256 + j, 0, n9]
            tb[:, :, q] = (
                t8[:, :, n9, j, 1, 0] * g_odd
            ) / (t8[:, :, n9, j, 0, 0] * g_even)
            tb[:, :, 4 + q] = (
                t8[:, :, n9, j, 1, 1] * g_odd
            ) / (t8[:, :, n9, j, 0, 1] * g_even)
    for n8 in range(2):
        for q9 in range(2):
            for n7 in range(2):
                j9 = n8 * 256 + n7 * 128 + pr
                tb[:, :, 8 + n8 * 4 + q9 * 2 + n7] = (
                    t9[:, :, j9, 1, q9] / t9[:, :, j9, 0, q9]
                )
    return S, tb


def _prep_host(x, twiddle, bias):
    """All host-side layout work. Returns per-core input maps (numpy)."""
    perm = _bitrev(N)
    y = np.ascontiguousarray(x).reshape(IC, B, N)[:, :, perm]
    # device layout y[ic, p, c*64+b], chunk-major c=(n9,n8,n7)
    y_dev = np.ascontiguousarray(
        y.reshape(IC, B, NCH, P).transpose(0, 3, 2, 1)
    ).reshape(IC, P, NCH * B)

    tw = np.asarray(twiddle, dtype=np.float64)
    A = _compose_stageA(tw)
    S, tb = _fold_and_ratios(tw)
    # scale A rows: block g=(n9,n8), row (n7out*128+p) *= S[n9,n8,n7out,p]
    Av = A.reshape(IC, OC, 2, 2, 2, P, SB)  # [n9, n8, n7out, p, col]
    Av *= S[..., None]

    bias_np = np.asarray(bias, dtype=np.float64).reshape(OC, NCH, P)

    in_maps = []
    for core in range(NCORES):
        osl = slice(core * OCL, (core + 1) * OCL)
        Ac = A[:, osl]  # (IC, OCL, 4, 256, 256) float64
        # lhsT tiles: w[ic,o,p_k, g, h, kin, m] = Ac[ic,o,g][h*128+m, kin*128+p_k]
        w = np.ascontiguousarray(
            Ac.reshape(IC, OCL, NBLK, 2, P, 2, P)  # [g, h, m, kin, k]
            .transpose(0, 1, 6, 2, 3, 5, 4)        # [ic,o,k,g,h,kin,m]
        ).astype(W_DTYPE).reshape(IC, OCL, P, WCOLS)
        tbc = np.ascontiguousarray(
            tb[:, osl].transpose(0, 1, 3, 2)  # [ic,o,p,16]
        ).astype(W_DTYPE)
        # resident table: tball[p, (ic*OCL+o)*16 + j]
        tball = np.ascontiguousarray(
            tbc.transpose(2, 0, 1, 3)
        ).reshape(P, IC * OCL * TBCOLS)
        # bias in acc layout (p9, n8, n7) = natural chunk order
        bc = np.ascontiguousarray(
            np.broadcast_to(
                bias_np[osl].transpose(0, 2, 1)[:, :, :, None],
                (OCL, P, NCH, B),
            )
        ).reshape(OCL, P, NCH * B).astype(W_DTYPE)
        in_maps.append(
            {
                "y": y_dev.astype(W_DTYPE),
                "w": w,
                "tb": tball,
                "bias": bc,
                "eye": np.eye(P, dtype=np.float32).astype(W_DTYPE),
            }
        )
    return in_maps


def _emulate_core(im):
    """Numpy emulation mirroring the device program (incl. bf16 rounding)."""
    f32 = np.float32

    def rt(a):  # round-trip through W_DTYPE
        return a.astype(W_DTYPE).astype(f32)

    y = im["y"].astype(f32)             # (IC, 128, 512)
    w = im["w"].astype(f32).reshape(IC, OCL, P, NBLK, 2, 2, P)
    tb = (
        im["tb"].astype(f32)
        .reshape(P, IC, OCL, TBCOLS)
        .transpose(1, 2, 0, 3)
    )  # (IC, OCL, P, 16)
    acc = im["bias"].astype(f32).reshape(OCL, P, 2, 2, 2, B).copy()
    for o in range(OCL):
        for ic in range(IC):
            z = np.zeros((P, 2, 2, 2, B), dtype=f32)  # [p, n8, n9, n7, b]
            yv = y[ic].reshape(P, NCH, B)  # chunks (n9,n8,n7)
            for g in range(NBLK):          # g = (n9, n8)
                n9, n8 = g >> 1, g & 1
                for h in range(2):         # n7out
                    a = np.zeros((P, B), dtype=f32)
                    for kin in range(2):
                        lhsT = w[ic, o, :, g, h, kin]  # [k, m]
                        a += lhsT.T @ yv[:, 2 * g + kin]
                    z[:, n8, n9, h] = a
            zc = rt(z)
            # L8: y8[p8, n9, n7]
            y8 = np.zeros_like(zc)
            y8[:, 0] = rt(zc[:, 0] + zc[:, 1])
            mm = rt(zc * tb[ic, o, :, :8].reshape(P, 2, 2, 2, 1))
            y8[:, 1] = rt(mm[:, 0] + mm[:, 1])
            # L9: u = y8 * (rA|rB)
            u = rt(y8 * tb[ic, o, :, 8:].reshape(P, 2, 2, 2, 1))
            # identity matmuls accumulate into fp32 acc [p9, n8, n7]
            acc[o, :, 0] += y8[:, :, 0] + y8[:, :, 1]
            acc[o, :, 1] += u[:, :, 0] + u[:, :, 1]
    return acc.reshape(OCL, P, NCH * B)


def _build_program():
    import concourse.bacc as bacc
    import concourse.mybir as mybir
    from concourse.tile import TileContext

    wdt = mybir.dt.bfloat16 if W_DTYPE != np.float32 else mybir.dt.float32
    f32 = mybir.dt.float32
    ADD, MULT = mybir.AluOpType.add, mybir.AluOpType.mult

    nc = bacc.Bacc(None, target_bir_lowering=False)
    y_d = nc.dram_tensor("y", (IC, P, NCH * B), wdt, kind="ExternalInput")
    w_d = nc.dram_tensor("w", (IC, OCL, P, WCOLS), wdt, kind="ExternalInput")
    tb_d = nc.dram_tensor(
        "tb", (P, IC * OCL * TBCOLS), wdt, kind="ExternalInput"
    )
    bias_d = nc.dram_tensor("bias", (OCL, P, NCH * B), wdt, kind="ExternalInput")
    eye_d = nc.dram_tensor("eye", (P, P), wdt, kind="ExternalInput")
    o_d = nc.dram_tensor("o", (OCL, P, NCH * B), f32, kind="ExternalOutput")

    with TileContext(nc) as tc:
        with (
            tc.tile_pool(name="ypool", bufs=2) as ypool,
            tc.tile_pool(name="wpool", bufs=4) as wpool,
            tc.tile_pool(name="const", bufs=1) as cpool,
            tc.tile_pool(name="zcpool", bufs=4) as zcpool,
            tc.tile_pool(name="y8pool", bufs=3 + DELAY_PAIRS) as y8pool,
            tc.tile_pool(name="mpool", bufs=2) as mpool,
            tc.tile_pool(name="upool", bufs=3 + DELAY_PAIRS) as upool,
            tc.tile_pool(name="rspool", bufs=3) as rspool,
            tc.tile_pool(name="opool", bufs=OCL) as opool,
            tc.tile_pool(name="zps", bufs=4, space="PSUM") as zps,
            tc.tile_pool(name="accps", bufs=OCL, space="PSUM") as accps,
        ):
            engs = {"V": nc.vector, "G": nc.gpsimd, "S": nc.scalar}

            def vec_copy(engine, out, in_):
                if engine == "S":
                    nc.scalar.copy(out, in_)
                else:
                    engs[engine].tensor_copy(out, in_)

            eye = cpool.tile([P, P], wdt, tag="eye")
            nc.sync.dma_start(out=eye[:], in_=eye_d[:, :])
            tball = cpool.tile([P, IC * OCL * TBCOLS], wdt, tag="tball")
            nc.sync.dma_start(out=tball[:], in_=tb_d[:, :])
            accs = []
            for o in range(OCL):
                bt = cpool.tile([P, NCH * B], wdt, tag=f"bias{o}")
                nc.sync.dma_start(out=bt[:], in_=bias_d[o])
                acc = accps.tile([P, NCH * B], f32, tag="acc")
                nc.tensor.matmul(
                    acc[:], eye[:], bt[:],
                    start=True, stop=False, skip_group_check=True,
                )
                accs.append(acc)

            pending = []

            def flush_one():
                o, y8t, ut, last = pending.pop(0)
                hB = NCH * B // 2
                split = "p (a q c b) -> p a q c b"
                if ACC_MM_MODE == "mm2":
                    # q9-sum via PSUM accumulate of a stride-0 out AP
                    for tile_, lo in ((y8t, True), (ut, False)):
                        rhs = tile_[:].rearrange(split, a=2, q=2, c=2)
                        base = accs[o][:, 0:hB] if lo else accs[o][:, hB:]
                        outap = (
                            base.rearrange("p (a c b) -> p a c b", a=2, c=2)
                            .unsqueeze(2)
                            .broadcast_to([P, 2, 2, 2, B])
                        )
                        nc.tensor.matmul(
                            outap,
                            eye[:],
                            rhs,
                            start=False,
                            stop=(last and not lo),
                            skip_group_check=True,
                        )
                else:
                    y8v = y8t[:].rearrange(split, a=2, q=2, c=2)
                    uv = ut[:].rearrange(split, a=2, q=2, c=2)
                    for k, rhs in enumerate(
                        (y8v[:, :, 0], y8v[:, :, 1], uv[:, :, 0], uv[:, :, 1])
                    ):
                        lo = k < 2
                        nc.tensor.matmul(
                            accs[o][:, 0:hB] if lo else accs[o][:, hB:],
                            eye[:],
                            rhs,
                            start=False,
                            stop=(last and k == 3),
                            skip_group_check=True,
                        )

            for ic in range(IC):
                ytile = ypool.tile([P, NCH * B], wdt)
                nc.sync.dma_start(out=ytile[:], in_=y_d[ic])
                for o in range(OCL):
                    wtile = wpool.tile([P, WCOLS], wdt)
                    nc.sync.dma_start(out=wtile[:], in_=w_d[ic, o])
                    z = zps.tile([P, 2, 2, 2, B], f32)  # [n8, n9, n7, b]
                    for g in range(NBLK):
                        n9, n8 = g >> 1, g & 1
                        for h in range(2):
                            for kin in range(2):
                                wi = ((g * 2 + h) * 2 + kin) * P
                                nc.tensor.matmul(
                                    z[:, n8, n9, h],
                                    wtile[:, wi : wi + P],
                                    ytile[:, (2 * g + kin) * B : (2 * g + kin + 1) * B],
                                    start=(kin == 0),
                                    stop=(kin == 1),
                                    skip_group_check=True,
                                )
                    while len(pending) >= DELAY_PAIRS:
                        flush_one()
                    tb0 = (ic * OCL + o) * TBCOLS
                    bcast8 = lambda c0: (
                        tball[:, tb0 + c0 : tb0 + c0 + 8]
                        .unsqueeze(2)
                        .broadcast_to([P, 8, B])
                    )
                    rs = rspool.tile([P, NCH * B], wdt, tag="rs")
                    vec_copy(
                        ENG["rs_mat"],
                        rs[:].rearrange("p (c b) -> p c b", c=8),
                        bcast8(0),
                    )
                    zc = zcpool.tile([P, 2, 2, 2, B], wdt, tag="zc")
                    vec_copy(ENG["zcopy"], zc[:], z[:])
                    zcf = zc[:].rearrange("p a b c d -> p (a b c d)")
                    hB = NCH * B // 2
                    y8 = y8pool.tile([P, NCH * B], wdt)  # [(p8, n9, n7), b]
                    m = mpool.tile([P, NCH * B], wdt)    # [(q8, n9, n7), b]
                    u = upool.tile([P, NCH * B], wdt)    # [(p8, q9, n7), b]
                    engs[ENG["l8low"]].tensor_tensor(
                        y8[:, 0:hB], zcf[:, 0:hB], zcf[:, hB:], op=ADD
                    )
                    engs[ENG["m_mult"]].tensor_tensor(
                        m[:], zcf, rs[:], op=MULT
                    )
                    engs[ENG["l8hi_add"]].tensor_tensor(
                        y8[:, hB:], m[:, 0:hB], m[:, hB:], op=ADD
                    )
                    engs[ENG["u_mult"]].tensor_tensor(
                        u[:].rearrange("p (c b) -> p c b", c=8),
                        y8[:].rearrange("p (c b) -> p c b", c=8),
                        bcast8(8),
                        op=MULT,
                    )
                    pending.append((o, y8, u, ic == IC - 1))
            while pending:
                flush_one()
            for o in range(OCL):
                ot = opool.tile([P, NCH * B], f32, tag=f"out{o}")
                nc.scalar.copy(ot[:], accs[o][:])
                nc.sync.dma_start(out=o_d[o], in_=ot[:])
    nc.finalize()
    return nc


_LAST_RESULTS = {"exec_time_ns": None}


def kernel(x, twiddle, bias, _trace=False, _emulate=False):
    in_maps = _prep_host(np.asarray(x), np.asarray(twiddle), np.asarray(bias))
    if _emulate:
        outs = [_emulate_core(im) for im in in_maps]
    else:
        from concourse.bass_utils import run_bass_kernel_spmd

        nc = _build_program()
        res = run_bass_kernel_spmd(
            nc, in_maps, list(range(NCORES)), trace=_trace
        )
        _LAST_RESULTS["exec_time_ns"] = res.exec_time_ns
        _LAST_RESULTS["mean_exec_time_ns"] = res.mean_exec_time_ns
        outs = [r["o"] for r in res.results]
    # o[oc_l, p, c*64+b] with chunk c=(n9,n8,n7) natural; (OC,B,N) ->
    # (B,OC,H,W) is a pure reinterpret (reference uses .reshape).
    full = np.concatenate(
        [
            np.asarray(o, dtype=np.float32)
            .reshape(OCL, P, NCH, B)
            .transpose(0, 3, 2, 1)
            .reshape(OCL, B, N)
            for o in outs
        ],
        axis=0,
    )
    return np.ascontiguousarray(full).reshape(B, OC, H, W).astype(np.float32)


# revision 19
# speedup vs baseline: 1.5229x; 1.0123x over previous
"""Butterfly-Conv2d (nn_BConv2d) Trainium2 kernel — v5.

Math (reference): x(B=64,IC=16,32,32) -> y=x.reshape(IC,B,N=1024)[:,:,bitrev];
broadcast over OC=32; 10 radix-2 butterfly layers with per-(ic,oc) twiddles;
mean over ic; + bias -> (B,OC,32,32).

Strategy:
  * Shard over OC: 8 cores x 4 oc each; every core holds all 16 ic so the
    ic-mean is core-local (no collective). Host concatenates oc slices.
  * Stage A (TensorE, bf16): butterfly layers 0..7 compose into dense
    256x256 block-diagonal matrices; K=128 matmuls with PSUM K-accumulation
    (16 MMs per (ic,oc), N=64). MMs pipeline at ~53 ns.
  * Ratio folding: for layers 8 and 9 the "low" output of each butterfly
    pair is made coefficient-free by folding its twiddle (and the 1/IC
    mean) into the stage-A matrix rows on the host; the "high" output uses
    host-computed coefficient RATIOS (bf16's 8-bit exponent keeps the
    Cauchy-tailed ratios safe — errors stay relative).
  * Stage B per (ic,oc) is four 2x-mode flat [128,512]/[128,256] DVE ops
    (one add + two coefficient mults + one add); the per-chunk coefficient
    vectors are materialized once per pair into flat tiles by ScalarE /
    GpSimd via stride-0 broadcast access patterns.
  * The remaining butterfly adds AND the ic-mean happen on TensorE: four
    identity matmuls per pair accumulate y8/u chunk-views straight into a
    per-oc PSUM accumulator (fp32, seeded by a bias matmul). These acc-MMs
    are issued DELAYED by 2 pairs so the in-order PE never stalls waiting
    for the current pair's vector chain.

Device layout: z/zc/y8/m/u tiles [128, (q8|p8, n9, n7, b)] with partition
p = n&127 (chunk-block order n8-major); acc [128, (p9, n8, n7, b)], which
makes the output chunk order natural.
"""

import numpy as np
import ml_dtypes

B, IC, OC, H, W = 64, 16, 32, 32, 32
N = H * W          # 1024
M = 10             # butterfly layers
NCORES = 8
OCL = OC // NCORES  # 4 oc per core
NCH = 8            # free-dim chunks (n9n8n7)
P = 128            # partitions (n6..n0)
SB = 256           # composed stage-A block size (layers 0..7)
NBLK = N // SB     # 4 blocks per (ic,oc)
WCOLS = NBLK * 4 * P      # 2048 weight cols per (ic,oc)
TBCOLS = 16               # ratio-table cols appended to w

W_DTYPE = ml_dtypes.bfloat16

# engine per stage-B op: 'V' = DVE, 'G' = GpSimd, 'S' = ScalarE.
# NOTE: GpSimd shares VectorE's SBUF port — concurrent GpSimd traffic
# slows DVE ops ~4x, so GpSimd is left idle on purpose.
ENG = {
    "zcopy": "S",      # zc = bf16(z), PSUM->SBUF [128,512]
    "rs_mat": "S",     # rs = bcast(tb cols 0..7)   [128,512]
    "l8low": "V",      # y8_low = zc0 + zc1          [128,256] add 2x
    "m_mult": "V",     # m = zc * rs                 [128,512] mult 2x
    "l8hi_add": "V",   # y8_hi = m0 + m1             [128,256] add 2x
    "u_mult": "V",     # u = y8 * (rA|rB)-bcast      [128,512] mult 1x
}
DELAY_PAIRS = 5        # acc-MM issue delay (pairs) to keep PE un-stalled
ACC_MM_MODE = "mm2"    # "mm2": stride-0-out accumulate MMs; "mm4": 4 plain MMs


def _bitrev(n):
    bits = int(np.log2(n))
    idx = np.arange(n, dtype=np.int64)
    rev = np.zeros(n, dtype=np.int64)
    for b in range(bits):
        rev = (rev << 1) | ((idx >> b) & 1)
    return rev


def _compose_stageA(tw):
    """Compose butterfly layers 0..7 into A[ic,oc,g,256,256] (g=(n9,n8))."""
    ic, oc = tw.shape[0], tw.shape[1]
    A = np.zeros((ic, oc, NBLK, SB, SB), dtype=np.float64)
    eye = np.eye(SB, dtype=np.float64)
    A[:] = eye
    for l in range(8):
        s = 1 << l
        nb_loc = SB // (2 * s)
        t = tw[:, :, l].reshape(ic, oc, N // (2 * s), s, 2, 2)
        t = t.reshape(ic, oc, NBLK, nb_loc, s, 2, 2).astype(np.float64)
        Av = A.reshape(ic, oc, NBLK, nb_loc, 2, s, SB)
        a0 = Av[:, :, :, :, 0].copy()
        a1 = Av[:, :, :, :, 1].copy()
        t00 = t[..., 0, 0, None]
        t01 = t[..., 0, 1, None]
        t10 = t[..., 1, 0, None]
        t11 = t[..., 1, 1, None]
        Av[:, :, :, :, 0] = t00 * a0 + t01 * a1
        Av[:, :, :, :, 1] = t10 * a0 + t11 * a1
    return A


def _fold_and_ratios(tw):
    """Row scalings for stage A plus stage-B ratio tables (float64).

    Returns (S, tb):
      S[ic, oc, n9, n8, n7, p]  row scaling for z-chunk (n9,n8,n7)
      tb[ic, oc, 16, p]  cols 0..7: (r|s) for L8-high, col = q8*4+n9*2+n7
                         cols 8..15: (rA|rB) for L9-high, col = 8+p8*4+q9*2+n7
    """
    ic, oc = tw.shape[0], tw.shape[1]
    # t8[k=n9, j(n7,p), p_out8, q8] with j in [0,256)
    t8 = tw[:, :, 8].reshape(ic, oc, 2, 256, 2, 2).astype(np.float64)
    t9 = tw[:, :, 9].reshape(ic, oc, 512, 2, 2).astype(np.float64)
    pr = np.arange(P)
    S = np.zeros((ic, oc, 2, 2, 2, P), dtype=np.float64)
    tb = np.zeros((ic, oc, TBCOLS, P), dtype=np.float64)
    for n9 in range(2):
        for n8 in range(2):
            for n7 in range(2):
                j = n7 * 128 + pr
                S[:, :, n9, n8, n7] = (
                    t8[:, :, n9, j, 0, n8] * t9[:, :, j, 0, n9] / IC
                )
    for n9 in range(2):
        for n7 in range(2):
            q = n9 * 2 + n7
            j = n7 * 128 + pr
            g_even = t9[:, :, 0 * 256 + j, 0, n9]
            g_odd = t9[:, :, 1 * 256 + j, 0, n9]
            tb[:, :, q] = (
                t8[:, :, n9, j, 1, 0] * g_odd
            ) / (t8[:, :, n9, j, 0, 0] * g_even)
            tb[:, :, 4 + q] = (
                t8[:, :, n9, j, 1, 1] * g_odd
            ) / (t8[:, :, n9, j, 0, 1] * g_even)
    for n8 in range(2):
        for q9 in range(2):
            for n7 in range(2):
                j9 = n8 * 256 + n7 * 128 + pr
                tb[:, :, 8 + n8 * 4 + q9 * 2 + n7] = (
                    t9[:, :, j9, 1, q9] / t9[:, :, j9, 0, q9]
                )
    return S, tb


def _prep_host(x, twiddle, bias):
    """All host-side layout work. Returns per-core input maps (numpy)."""
    perm = _bitrev(N)
    y = np.ascontiguousarray(x).reshape(IC, B, N)[:, :, perm]
    # device layout y[ic, p, c*64+b], chunk-major c=(n9,n8,n7)
    y_dev = np.ascontiguousarray(
        y.reshape(IC, B, NCH, P).transpose(0, 3, 2, 1)
    ).reshape(IC, P, NCH * B)

    tw = np.asarray(twiddle, dtype=np.float64)
    A = _compose_stageA(tw)
    S, tb = _fold_and_ratios(tw)
    # scale A rows: block g=(n9,n8), row (n7out*128+p) *= S[n9,n8,n7out,p]
    Av = A.reshape(IC, OC, 2, 2, 2, P, SB)  # [n9, n8, n7out, p, col]
    Av *= S[..., None]

    bias_np = np.asarray(bias, dtype=np.float64).reshape(OC, NCH, P)

    in_maps = []
    for core in range(NCORES):
        osl = slice(core * OCL, (core + 1) * OCL)
        Ac = A[:, osl]  # (IC, OCL, 4, 256, 256) float64
        # lhsT tiles: w[ic,o,p_k, g, h, kin, m] = Ac[ic,o,g][h*128+m, kin*128+p_k]
        w = np.ascontiguousarray(
            Ac.reshape(IC, OCL, NBLK, 2, P, 2, P)  # [g, h, m, kin, k]
            .transpose(0, 1, 6, 2, 3, 5, 4)        # [ic,o,k,g,h,kin,m]
        ).astype(W_DTYPE).reshape(IC, OCL, P, WCOLS)
        tbc = np.ascontiguousarray(
            tb[:, osl].transpose(0, 1, 3, 2)  # [ic,o,p,16]
        ).astype(W_DTYPE)
        # resident table: tball[p, (ic*OCL+o)*16 + j]
        tball = np.ascontiguousarray(
            tbc.transpose(2, 0, 1, 3)
        ).reshape(P, IC * OCL * TBCOLS)
        # bias in acc layout (p9, n8, n7) = natural chunk order
        bc = np.ascontiguousarray(
            np.broadcast_to(
                bias_np[osl].transpose(0, 2, 1)[:, :, :, None],
                (OCL, P, NCH, B),
            )
        ).reshape(OCL, P, NCH * B).astype(W_DTYPE)
        in_maps.append(
            {
                "y": y_dev.astype(W_DTYPE),
                "w": w,
                "tb": tball,
                "bias": bc,
                "eye": np.eye(P, dtype=np.float32).astype(W_DTYPE),
            }
        )
    return in_maps


def _emulate_core(im):
    """Numpy emulation mirroring the device program (incl. bf16 rounding)."""
    f32 = np.float32

    def rt(a):  # round-trip through W_DTYPE
        return a.astype(W_DTYPE).astype(f32)

    y = im["y"].astype(f32)             # (IC, 128, 512)
    w = im["w"].astype(f32).reshape(IC, OCL, P, NBLK, 2, 2, P)
    tb = (
        im["tb"].astype(f32)
        .reshape(P, IC, OCL, TBCOLS)
        .transpose(1, 2, 0, 3)
    )  # (IC, OCL, P, 16)
    acc = im["bias"].astype(f32).reshape(OCL, P, 2, 2, 2, B).copy()
    for o in range(OCL):
        for ic in range(IC):
            z = np.zeros((P, 2, 2, 2, B), dtype=f32)  # [p, n8, n9, n7, b]
            yv = y[ic].reshape(P, NCH, B)  # chunks (n9,n8,n7)
            for g in range(NBLK):          # g = (n9, n8)
                n9, n8 = g >> 1, g & 1
                for h in range(2):         # n7out
                    a = np.zeros((P, B), dtype=f32)
                    for kin in range(2):
                        lhsT = w[ic, o, :, g, h, kin]  # [k, m]
                        a += lhsT.T @ yv[:, 2 * g + kin]
                    z[:, n8, n9, h] = a
            zc = rt(z)
            # L8: y8[p8, n9, n7]
            y8 = np.zeros_like(zc)
            y8[:, 0] = rt(zc[:, 0] + zc[:, 1])
            mm = rt(zc * tb[ic, o, :, :8].reshape(P, 2, 2, 2, 1))
            y8[:, 1] = rt(mm[:, 0] + mm[:, 1])
            # L9: u = y8 * (rA|rB)
            u = rt(y8 * tb[ic, o, :, 8:].reshape(P, 2, 2, 2, 1))
            # identity matmuls accumulate into fp32 acc [p9, n8, n7]
            acc[o, :, 0] += y8[:, :, 0] + y8[:, :, 1]
            acc[o, :, 1] += u[:, :, 0] + u[:, :, 1]
    return acc.reshape(OCL, P, NCH * B)


def _build_program():
    import concourse.bacc as bacc
    import concourse.mybir as mybir
    from concourse.tile import TileContext

    wdt = mybir.dt.bfloat16 if W_DTYPE != np.float32 else mybir.dt.float32
    f32 = mybir.dt.float32
    ADD, MULT = mybir.AluOpType.add, mybir.AluOpType.mult

    nc = bacc.Bacc(None, target_bir_lowering=False)
    y_d = nc.dram_tensor("y", (IC, P, NCH * B), wdt, kind="ExternalInput")
    w_d = nc.dram_tensor("w", (IC, OCL, P, WCOLS), wdt, kind="ExternalInput")
    tb_d = nc.dram_tensor(
        "tb", (P, IC * OCL * TBCOLS), wdt, kind="ExternalInput"
    )
    bias_d = nc.dram_tensor("bias", (OCL, P, NCH * B), wdt, kind="ExternalInput")
    eye_d = nc.dram_tensor("eye", (P, P), wdt, kind="ExternalInput")
    o_d = nc.dram_tensor("o", (OCL, P, NCH * B), f32, kind="ExternalOutput")

    with TileContext(nc) as tc:
        with (
            tc.tile_pool(name="ypool", bufs=3) as ypool,
            tc.tile_pool(name="wpool", bufs=6) as wpool,
            tc.tile_pool(name="const", bufs=1) as cpool,
            tc.tile_pool(name="zcpool", bufs=6) as zcpool,
            tc.tile_pool(name="y8pool", bufs=3 + DELAY_PAIRS) as y8pool,
            tc.tile_pool(name="mpool", bufs=4) as mpool,
            tc.tile_pool(name="upool", bufs=3 + DELAY_PAIRS) as upool,
            tc.tile_pool(name="rspool", bufs=6) as rspool,
            tc.tile_pool(name="opool", bufs=OCL) as opool,
            tc.tile_pool(name="zps", bufs=4, space="PSUM") as zps,
            tc.tile_pool(name="accps", bufs=OCL, space="PSUM") as accps,
        ):
            engs = {"V": nc.vector, "G": nc.gpsimd, "S": nc.scalar}

            def vec_copy(engine, out, in_):
                if engine == "S":
                    nc.scalar.copy(out, in_)
                else:
                    engs[engine].tensor_copy(out, in_)

            eye = cpool.tile([P, P], wdt, tag="eye")
            nc.sync.dma_start(out=eye[:], in_=eye_d[:, :])
            tball = cpool.tile([P, IC * OCL * TBCOLS], wdt, tag="tball")
            nc.sync.dma_start(out=tball[:], in_=tb_d[:, :])
            accs = []
            for o in range(OCL):
                bt = cpool.tile([P, NCH * B], wdt, tag=f"bias{o}")
                nc.sync.dma_start(out=bt[:], in_=bias_d[o])
                acc = accps.tile([P, NCH * B], f32, tag="acc")
                nc.tensor.matmul(
                    acc[:], eye[:], bt[:],
                    start=True, stop=False, skip_group_check=True,
                )
                accs.append(acc)

            pending = []

            def flush_one():
                o, y8t, ut, last = pending.pop(0)
                hB = NCH * B // 2
                split = "p (a q c b) -> p a q c b"
                if ACC_MM_MODE == "mm2":
                    # q9-sum via PSUM accumulate of a stride-0 out AP
                    for tile_, lo in ((y8t, True), (ut, False)):
                        rhs = tile_[:].rearrange(split, a=2, q=2, c=2)
                        base = accs[o][:, 0:hB] if lo else accs[o][:, hB:]
                        outap = (
                            base.rearrange("p (a c b) -> p a c b", a=2, c=2)
                            .unsqueeze(2)
                            .broadcast_to([P, 2, 2, 2, B])
                        )
                        nc.tensor.matmul(
                            outap,
                            eye[:],
                            rhs,
                            start=False,
                            stop=(last and not lo),
                            skip_group_check=True,
                        )
                else:
                    y8v = y8t[:].rearrange(split, a=2, q=2, c=2)
                    uv = ut[:].rearrange(split, a=2, q=2, c=2)
                    for k, rhs in enumerate(
                        (y8v[:, :, 0], y8v[:, :, 1], uv[:, :, 0], uv[:, :, 1])
                    ):
                        lo = k < 2
                        nc.tensor.matmul(
                            accs[o][:, 0:hB] if lo else accs[o][:, hB:],
                            eye[:],
                            rhs,
                            start=False,
                            stop=(last and k == 3),
                            skip_group_check=True,
                        )

            for ic in range(IC):
                ytile = ypool.tile([P, NCH * B], wdt)
                nc.sync.dma_start(out=ytile[:], in_=y_d[ic])
                for o in range(OCL):
                    wtile = wpool.tile([P, WCOLS], wdt)
                    nc.sync.dma_start(out=wtile[:], in_=w_d[ic, o])
                    z = zps.tile([P, 2, 2, 2, B], f32)  # [n8, n9, n7, b]
                    for g in range(NBLK):
                        n9, n8 = g >> 1, g & 1
                        for h in range(2):
                            for kin in range(2):
                                wi = ((g * 2 + h) * 2 + kin) * P
                                nc.tensor.matmul(
                                    z[:, n8, n9, h],
                                    wtile[:, wi : wi + P],
                                    ytile[:, (2 * g + kin) * B : (2 * g + kin + 1) * B],
                                    start=(kin == 0),
                                    stop=(kin == 1),
                                    skip_group_check=True,
                                )
                    while len(pending) >= DELAY_PAIRS:
                        flush_one()
                    tb0 = (ic * OCL + o) * TBCOLS
                    bcast8 = lambda c0: (
                        tball[:, tb0 + c0 : tb0 + c0 + 8]
                        .unsqueeze(2)
                        .broadcast_to([P, 8, B])
                    )
                    zc = zcpool.tile([P, 2, 2, 2, B], wdt, tag="zc")
                    vec_copy(ENG["zcopy"], zc[:], z[:])
                    rs = rspool.tile([P, NCH * B], wdt, tag="rs")
                    vec_copy(
                        ENG["rs_mat"],
                        rs[:].rearrange("p (c b) -> p c b", c=8),
                        bcast8(0),
                    )
                    zcf = zc[:].rearrange("p a b c d -> p (a b c d)")
                    hB = NCH * B // 2
                    y8 = y8pool.tile([P, NCH * B], wdt)  # [(p8, n9, n7), b]
                    m = mpool.tile([P, NCH * B], wdt)    # [(q8, n9, n7), b]
                    u = upool.tile([P, NCH * B], wdt)    # [(p8, q9, n7), b]
                    engs[ENG["l8low"]].tensor_tensor(
                        y8[:, 0:hB], zcf[:, 0:hB], zcf[:, hB:], op=ADD
                    )
                    engs[ENG["m_mult"]].tensor_tensor(
                        m[:], zcf, rs[:], op=MULT
                    )
                    engs[ENG["l8hi_add"]].tensor_tensor(
                        y8[:, hB:], m[:, 0:hB], m[:, hB:], op=ADD
                    )
                    engs[ENG["u_mult"]].tensor_tensor(
                        u[:].rearrange("p (c b) -> p c b", c=8),
                        y8[:].rearrange("p (c b) -> p c b", c=8),
                        bcast8(8),
                        op=MULT,
                    )
                    pending.append((o, y8, u, ic == IC - 1))
            while pending:
                flush_one()
            for o in range(OCL):
                ot = opool.tile([P, NCH * B], f32, tag=f"out{o}")
                nc.scalar.copy(ot[:], accs[o][:])
                nc.sync.dma_start(out=o_d[o], in_=ot[:])
    nc.finalize()
    return nc


_LAST_RESULTS = {"exec_time_ns": None}


def kernel(x, twiddle, bias, _trace=False, _emulate=False):
    in_maps = _prep_host(np.asarray(x), np.asarray(twiddle), np.asarray(bias))
    if _emulate:
        outs = [_emulate_core(im) for im in in_maps]
    else:
        from concourse.bass_utils import run_bass_kernel_spmd

        nc = _build_program()
        res = run_bass_kernel_spmd(
            nc, in_maps, list(range(NCORES)), trace=_trace
        )
        _LAST_RESULTS["exec_time_ns"] = res.exec_time_ns
        _LAST_RESULTS["mean_exec_time_ns"] = res.mean_exec_time_ns
        outs = [r["o"] for r in res.results]
    # o[oc_l, p, c*64+b] with chunk c=(n9,n8,n7) natural; (OC,B,N) ->
    # (B,OC,H,W) is a pure reinterpret (reference uses .reshape).
    full = np.concatenate(
        [
            np.asarray(o, dtype=np.float32)
            .reshape(OCL, P, NCH, B)
            .transpose(0, 3, 2, 1)
            .reshape(OCL, B, N)
            for o in outs
        ],
        axis=0,
    )
    return np.ascontiguousarray(full).reshape(B, OC, H, W).astype(np.float32)
